# revision 14
# baseline (speedup 1.0000x reference)
"""Trainium2 Bass kernel: multi-head self-attention with RoPE + causal mask.

Problem shapes (hardcoded): x [2, 2048, 2048] f32, wq/wk/wv/wo [2048, 2048] f32.
  D_MODEL=2048, NUM_HEADS=16, D_K=128, SEQ=2048, BATCH=2, THETA=1e4.

Sharding: tensor-parallel over heads. Each of the 8 cores computes 2 heads:
  - q/k/v projections for its head slice (wq/wk/wv rows 256c:256c+256),
  - RoPE, causal attention (scoresT layout: kv position on the partition dim),
  - its partial output projection (wo columns 256c:256c+256).
The 8 partial outputs [4096, 2048] are summed on the host (the "all-reduce").

Per-core kernel layout choices:
  - qT/kT per head: [dk=128 partitions, 4096 tokens] in SBUF.
  - scoresT tile [128 s, 512 q] = kT_tile.T-free matmul, so the attention
    context accumulates with N=512 matmuls (lhsT = v tile [128 s, 128 dv]).
  - softmax denominator accumulated on PE via a ones-column matmul into a
    [1, 512] psum; reciprocal on DVE; broadcast across partitions on GPSIMD.
  - causal mask: additive -1e5 mask on diagonal 128x128 blocks before exp.
All matmul inputs bf16 (fp32 matmul is 4x slower on TRN2 PE); accumulation f32.
"""

import math
from contextlib import ExitStack

import numpy as np
import ml_dtypes

import concourse.bass as bass
import concourse.tile as tile
from concourse import bacc, mybir
from concourse.bass_utils import run_bass_kernel_spmd

B, S, D, H, DK = 2, 2048, 2048, 16, 128
T = B * S              # 4096 flattened tokens
NCORES = 8
HPC = H // NCORES      # 2 heads per core
DH = HPC * DK          # 256 = per-core head-slice width
THETA = 10000.0
SCALE = 1.0 / math.sqrt(DK)
NEGMASK = -1.0e5

BF16 = mybir.dt.bfloat16
F32 = mybir.dt.float32
EXP = mybir.ActivationFunctionType.Exp

NB = S // 512          # q-blocks per batch element (4)
NJB = T // 512         # token blocks overall (8)
NTT = T // 128         # token tiles overall (32)


def _emit(ctx: ExitStack, tc: tile.TileContext, io: dict):
    nc = tc.nc
    xT, wqT, wkT, wvT, woT = io["xT"], io["wqT"], io["wkT"], io["wvT"], io["woT"]
    cosT, sinT, rT, negm, onesc, out = (
        io["cosT"], io["sinT"], io["rT"], io["negm"], io["onesc"], io["out"])

    const = ctx.enter_context(tc.tile_pool(name="const", bufs=1))
    pers = ctx.enter_context(tc.tile_pool(name="pers", bufs=1))

    # ---- constants into SBUF ----
    wq_sb = const.tile([128, 16, DH], BF16, name="wq_sb", tag="wq_sb")
    nc.sync.dma_start(out=wq_sb, in_=wqT.rearrange("(n p) k -> p n k", p=128))
    wk_sb = const.tile([128, 16, DH], BF16, name="wk_sb", tag="wk_sb")
    nc.sync.dma_start(out=wk_sb, in_=wkT.rearrange("(n p) k -> p n k", p=128))
    wv_sb = const.tile([128, 16, DH], BF16, name="wv_sb", tag="wv_sb")
    nc.sync.dma_start(out=wv_sb, in_=wvT.rearrange("(n p) k -> p n k", p=128))
    wo_sb = const.tile([128, HPC, D], BF16, name="wo_sb", tag="wo_sb")
    nc.sync.dma_start(out=wo_sb, in_=woT.rearrange("(h p) d -> p h d", p=128))
    cos_sb = const.tile([128, S], F32, name="cos_sb", tag="cos_sb")
    nc.sync.dma_start(out=cos_sb, in_=cosT)
    sin_sb = const.tile([128, S], F32, name="sin_sb", tag="sin_sb")
    nc.sync.dma_start(out=sin_sb, in_=sinT)
    rT_sb = const.tile([128, 128], BF16, name="rT_sb", tag="rT_sb")
    nc.sync.dma_start(out=rT_sb, in_=rT)
    negm_sb = const.tile([128, 128], F32, name="negm_sb", tag="negm_sb")
    nc.sync.dma_start(out=negm_sb, in_=negm)
    ones_sb = const.tile([128, 1], BF16, name="ones_sb", tag="ones_sb")
    nc.sync.dma_start(out=ones_sb, in_=onesc)
    onesr_sb = const.tile([1, 128], mybir.dt.float32r, name="onesr_sb",
                          tag="onesr_sb")
    nc.sync.dma_start(out=onesr_sb, in_=io["onesr"])

    # ---- persistent per-core activation tensors ----
    qT_sb = pers.tile([128, HPC, T], BF16, name="qT_sb", tag="qT_sb")
    kT_sb = pers.tile([128, HPC, T], BF16, name="kT_sb", tag="kT_sb")
    v_sb = pers.tile([128, HPC, NTT, DK], BF16, name="v_sb", tag="v_sb")
    ctxT_sb = pers.tile([128, HPC, T], BF16, name="ctxT_sb", tag="ctxT_sb")

    # ================= Phase 1: q/k/v projections + RoPE =================
    with (
        tc.tile_pool(name="xp", bufs=2) as xp,
        tc.tile_pool(name="p1t", bufs=3) as p1t,
        tc.tile_pool(name="pp_proj", bufs=2, space="PSUM") as pp_proj,
        tc.tile_pool(name="pp_rot", bufs=2, space="PSUM") as pp_rot,
        tc.tile_pool(name="pp_v", bufs=2, space="PSUM") as pp_v,
    ):
        xT_r = xT.rearrange("(n p) t -> p n t", p=128)
        for jb in range(NJB):
            x_t = xp.tile([128, 16, 512], BF16, name="x_t", tag="x_t")
            nc.sync.dma_start(out=x_t, in_=xT_r[:, :, jb * 512:(jb + 1) * 512])
            sblk = (jb % NB) * 512  # position within the batch element
            cos_b = cos_sb[:, sblk:sblk + 512]
            sin_b = sin_sb[:, sblk:sblk + 512]
            tb = slice(jb * 512, (jb + 1) * 512)
            for h in range(HPC):
                hs = slice(h * 128, (h + 1) * 128)
                for w_sb, dstT in ((wq_sb, qT_sb), (wk_sb, kT_sb)):
                    ps = pp_proj.tile([128, 512], F32, name="proj_ps", tag="proj_ps")
                    for n in range(16):
                        nc.tensor.matmul(ps, w_sb[:, n, hs], x_t[:, n, :],
                                         start=(n == 0), stop=(n == 15))
                    raw = p1t.tile([128, 512], BF16, name="raw", tag="raw")
                    nc.vector.tensor_copy(out=raw, in_=ps)
                    rot = pp_rot.tile([128, 512], F32, name="rot_ps", tag="rot_ps")
                    nc.tensor.matmul(rot, rT_sb, raw, start=True, stop=True)
                    t1 = p1t.tile([128, 512], F32, name="t1", tag="t1")
                    nc.vector.tensor_mul(out=t1, in0=ps, in1=cos_b)
                    t2 = p1t.tile([128, 512], F32, name="t2", tag="t2")
                    nc.vector.tensor_mul(out=t2, in0=rot, in1=sin_b)
                    nc.vector.tensor_add(out=dstT[:, h, tb], in0=t1, in1=t2)
            for tj in range(4):
                vps = pp_v.tile([128, DH], F32, name="v_ps", tag="v_ps")
                for n in range(16):
                    nc.tensor.matmul(vps, x_t[:, n, tj * 128:(tj + 1) * 128],
                                     wv_sb[:, n, :], start=(n == 0), stop=(n == 15))
                nc.vector.tensor_copy(
                    out=v_sb[:, :, jb * 4 + tj, :],
                    in_=vps.rearrange("p (h k) -> p h k", h=HPC))

    # ================= Phase 2: causal attention per (b, h, q-block) =====
    with (
        tc.tile_pool(name="expp", bufs=3) as expp,
        tc.tile_pool(name="rdp", bufs=2) as rdp,
        tc.tile_pool(name="rbp", bufs=2) as rbp,
        tc.tile_pool(name="ps_sc", bufs=2, space="PSUM") as ps_sc,
        tc.tile_pool(name="ps_ctx", bufs=2, space="PSUM") as ps_ctx,
        tc.tile_pool(name="ps_den", bufs=2, space="PSUM") as ps_den,
        tc.tile_pool(name="ps_bc", bufs=2, space="PSUM") as ps_bc,
    ):
        F32R = mybir.dt.float32r
        for b in range(B):
            for h in range(HPC):
                for qb in range(NB):
                    qg = b * S + qb * 512
                    ctx_ps = ps_ctx.tile([128, 512], F32, name="ctx_ps", tag="ctx_ps")
                    den_ps = ps_den.tile([1, 512], F32, name="den_ps", tag="den_ps")
                    nts = 4 * qb + 4
                    for ts in range(nts):
                        j = ts - 4 * qb  # >=0 on diagonal tiles
                        qoff = 128 * j if j >= 0 else 0
                        n_q = 512 - qoff
                        sg = b * S + ts * 128
                        sc = ps_sc.tile([128, 512], F32, name="sc_ps", tag="sc_ps")
                        nc.tensor.matmul(
                            sc[:, :n_q],
                            kT_sb[:, h, sg:sg + 128],
                            qT_sb[:, h, qg + qoff:qg + 512],
                            start=True, stop=True)
                        if j >= 0:
                            nc.vector.tensor_add(
                                out=sc[:, :128], in0=sc[:, :128], in1=negm_sb)
                        ex = expp.tile([128, 512], BF16, name="ex", tag="ex")
                        nc.scalar.activation(
                            out=ex[:, :n_q], in_=sc[:, :n_q], func=EXP, scale=SCALE)
                        nc.tensor.matmul(
                            den_ps[:, qoff:], ones_sb, ex[:, :n_q],
                            start=(ts == 0), stop=(ts == nts - 1))
                        nc.tensor.matmul(
                            ctx_ps[:, qoff:], v_sb[:, h, b * 16 + ts, :], ex[:, :n_q],
                            start=(ts == 0), stop=(ts == nts - 1))
                    rden = rdp.tile([1, 512], F32, name="rden", tag="rden")
                    nc.vector.reciprocal_approx_fast(out=rden, in_=den_ps)
                    rden_r = rdp.tile([1, 512], F32R, name="rden_r", tag="rden_r")
                    nc.vector.tensor_copy(out=rden_r, in_=rden)
                    # broadcast 1/denom across partitions: outer product on PE
                    bc_ps = ps_bc.tile([128, 512], F32, name="bc_ps", tag="bc_ps")
                    nc.tensor.matmul(bc_ps, onesr_sb, rden_r,
                                     start=True, stop=True)
                    rbc = rbp.tile([128, 512], F32, name="rbc", tag="rbc")
                    nc.scalar.copy(out=rbc, in_=bc_ps)
                    nc.vector.tensor_mul(
                        out=ctxT_sb[:, h, qg:qg + 512], in0=ctx_ps, in1=rbc)

    # ================= Phase 3: output projection (partial sum) ==========
    with (
        tc.tile_pool(name="outp", bufs=3) as outp,
        tc.tile_pool(name="ps_o", bufs=4, space="PSUM") as ps_o,
    ):
        for i in range(NTT):
            ot = outp.tile([128, D], BF16, name="ot", tag="ot")
            for jn in range(4):
                ops_t = ps_o.tile([128, 512], F32, name="ops", tag="ops")
                for h in range(HPC):
                    nc.tensor.matmul(
                        ops_t,
                        ctxT_sb[:, h, i * 128:(i + 1) * 128],
                        wo_sb[:, h, jn * 512:(jn + 1) * 512],
                        start=(h == 0), stop=(h == HPC - 1))
                nc.vector.tensor_copy(
                    out=ot[:, jn * 512:(jn + 1) * 512], in_=ops_t)
            nc.sync.dma_start(out=out[i * 128:(i + 1) * 128, :], in_=ot)


def build_bass():
    nc = bacc.Bacc("TRN2", target_bir_lowering=False, debug=False)
    io = {}

    def din(name, shape, dt):
        io[name] = nc.dram_tensor(name, list(shape), dt, kind="ExternalInput").ap()

    din("xT", (D, T), BF16)
    din("wqT", (D, DH), BF16)
    din("wkT", (D, DH), BF16)
    din("wvT", (D, DH), BF16)
    din("woT", (DH, D), BF16)
    din("cosT", (DK, S), F32)
    din("sinT", (DK, S), F32)
    din("rT", (DK, DK), BF16)
    din("negm", (128, 128), F32)
    din("onesc", (128, 1), BF16)
    din("onesr", (1, 128), mybir.dt.float32r)
    io["out"] = nc.dram_tensor("out", [T, D], BF16, kind="ExternalOutput").ap()

    with tile.TileContext(nc) as tc:
        with ExitStack() as ctx:
            _emit(ctx, tc, io)
    nc.compile()
    return nc


def host_tables():
    bf16 = ml_dtypes.bfloat16
    pos = np.arange(S, dtype=np.float64)
    inv = THETA ** (-(np.arange(0, DK, 2, dtype=np.float64) / DK))
    ang = pos[:, None] * inv[None, :]          # [S, 64]
    cosT = np.repeat(np.cos(ang).T, 2, axis=0).astype(np.float32)  # [128, S]
    sinT = np.repeat(np.sin(ang).T, 2, axis=0).astype(np.float32)
    # rotate-half operator: rot = R @ q with rot[2j] = -q[2j+1], rot[2j+1] = q[2j]
    R = np.zeros((DK, DK), np.float32)
    jj = np.arange(0, DK, 2)
    R[jj, jj + 1] = -1.0
    R[jj + 1, jj] = 1.0
    rT = np.ascontiguousarray(R.T).astype(bf16)
    sq = np.arange(128)
    negm = np.where(sq[:, None] > sq[None, :], np.float32(NEGMASK),
                    np.float32(0.0)).astype(np.float32)
    onesc = np.ones((128, 1), bf16)
    onesr = np.ones((1, 128), np.float32)
    return cosT, sinT, rT, negm, onesc, onesr


def make_in_maps(x, wq, wk, wv, wo):
    bf16 = ml_dtypes.bfloat16
    x = np.asarray(x, np.float32)
    wq, wk, wv, wo = (np.asarray(a, np.float32) for a in (wq, wk, wv, wo))
    xT = np.ascontiguousarray(x.reshape(T, D).T).astype(bf16)
    cosT, sinT, rT, negm, onesc, onesr = host_tables()
    in_maps = []
    for c in range(NCORES):
        hs = slice(c * DH, (c + 1) * DH)
        in_maps.append({
            "xT": xT,
            "wqT": np.ascontiguousarray(wq[hs, :].T).astype(bf16),
            "wkT": np.ascontiguousarray(wk[hs, :].T).astype(bf16),
            "wvT": np.ascontiguousarray(wv[hs, :].T).astype(bf16),
            "woT": np.ascontiguousarray(wo[:, hs].T).astype(bf16),
            "cosT": cosT,
            "sinT": sinT,
            "rT": rT,
            "negm": negm,
            "onesc": onesc,
            "onesr": onesr,
        })
    return in_maps


_CACHE = {}


def get_bass():
    if "nc" not in _CACHE:
        _CACHE["nc"] = build_bass()
    return _CACHE["nc"]


def run(inputs: dict, trace: bool = False):
    """Run on 8 NeuronCores; returns (full_output [B,S,D] f32, BassKernelResults)."""
    nc = get_bass()
    in_maps = make_in_maps(**inputs)
    res = run_bass_kernel_spmd(
        nc, in_maps, core_ids=list(range(NCORES)), trace=trace)
    acc = np.zeros((T, D), np.float32)
    for r in res.results:
        acc += np.asarray(r["out"], dtype=np.float32)
    return acc.reshape(B, S, D), res


def kernel(x, wq, wk, wv, wo):
    out, _ = run(dict(x=x, wq=wq, wk=wk, wv=wv, wo=wo))
    return out


# revision 15
# speedup vs baseline: 1.0575x; 1.0575x over previous
"""Trainium2 Bass kernel: multi-head self-attention with RoPE + causal mask.

Problem shapes (hardcoded): x [2, 2048, 2048] f32, wq/wk/wv/wo [2048, 2048] f32.
  D_MODEL=2048, NUM_HEADS=16, D_K=128, SEQ=2048, BATCH=2, THETA=1e4.

Sharding: tensor-parallel over heads. Each of the 8 cores computes 2 heads:
  - q/k/v projections for its head slice (wq/wk/wv rows 256c:256c+256),
  - RoPE, causal attention (scoresT layout: kv position on the partition dim),
  - its partial output projection (wo columns 256c:256c+256).
The 8 partial outputs [4096, 2048] are summed on the host (the "all-reduce").

Per-core kernel layout choices:
  - qT/kT per head: [dk=128 partitions, 4096 tokens] in SBUF.
  - scoresT tile [128 s, 512 q] so the attention context accumulates with
    N=512 matmuls (lhsT = v tile [128 s, 128 dv], rhs = exp tile).
  - softmax denominator: DVE-accumulated exp sums, reduced across partitions
    with a single f32r ones-column matmul per q-block; reciprocal on DVE;
    broadcast back across partitions via an f32r outer-product matmul.
  - causal mask: additive -1e5 on diagonal 128x128 blocks before exp.
  - output projection interleaved with attention per (batch, q-block) so PE
    has independent work while ScalarE computes exps.
All matmul inputs bf16 (fp32 matmul is 4x slower on TRN2 PE); accumulation f32.
"""

import math
from contextlib import ExitStack

import numpy as np
import ml_dtypes

import concourse.bass as bass
import concourse.tile as tile
from concourse import bacc, mybir
from concourse.bass_utils import run_bass_kernel_spmd

B, S, D, H, DK = 2, 2048, 2048, 16, 128
T = B * S              # 4096 flattened tokens
NCORES = 8
HPC = H // NCORES      # 2 heads per core
DH = HPC * DK          # 256 = per-core head-slice width
THETA = 10000.0
SCALE = 1.0 / math.sqrt(DK)
NEGMASK = -1.0e5

BF16 = mybir.dt.bfloat16
F32 = mybir.dt.float32
F32R = mybir.dt.float32r
EXP = mybir.ActivationFunctionType.Exp

NB = S // 512          # q-blocks per batch element (4)
NJB = T // 512         # token blocks overall (8)
NTT = T // 128         # token tiles overall (32)


def _emit(ctx: ExitStack, tc: tile.TileContext, io: dict):
    nc = tc.nc
    xT, wqT, wkT, wvT, woT = io["xT"], io["wqT"], io["wkT"], io["wvT"], io["woT"]
    out = io["out"]

    const = ctx.enter_context(tc.tile_pool(name="const", bufs=1))
    pers = ctx.enter_context(tc.tile_pool(name="pers", bufs=1))

    # ---- constants into SBUF (chunked so consumers can start early) ----
    def load_w(name, src):
        t = const.tile([128, 16, DH], BF16, name=name, tag=name)
        r = src.rearrange("(n p) k -> p n k", p=128)
        for c in range(4):
            nc.sync.dma_start(out=t[:, 4 * c:4 * c + 4, :],
                              in_=r[:, 4 * c:4 * c + 4, :])
        return t

    wq_sb = load_w("wq_sb", wqT)
    wk_sb = load_w("wk_sb", wkT)
    wv_sb = load_w("wv_sb", wvT)
    cos_sb = const.tile([128, S], F32, name="cos_sb", tag="cos_sb")
    nc.sync.dma_start(out=cos_sb, in_=io["cosT"])
    sin_sb = const.tile([128, S], F32, name="sin_sb", tag="sin_sb")
    nc.sync.dma_start(out=sin_sb, in_=io["sinT"])
    rT_sb = const.tile([128, 128], BF16, name="rT_sb", tag="rT_sb")
    nc.sync.dma_start(out=rT_sb, in_=io["rT"])
    negm_sb = const.tile([128, 128], F32, name="negm_sb", tag="negm_sb")
    nc.sync.dma_start(out=negm_sb, in_=io["negm"])
    onescr_sb = const.tile([128, 1], F32R, name="onescr_sb", tag="onescr_sb")
    nc.sync.dma_start(out=onescr_sb, in_=io["onescr"])
    onesr_sb = const.tile([1, 128], F32R, name="onesr_sb", tag="onesr_sb")
    nc.sync.dma_start(out=onesr_sb, in_=io["onesr"])
    wo_sb = const.tile([128, HPC, D], BF16, name="wo_sb", tag="wo_sb")
    nc.sync.dma_start(out=wo_sb, in_=woT.rearrange("(h p) d -> p h d", p=128))

    # ---- persistent per-core activation tensors ----
    qT_sb = pers.tile([128, HPC, T], BF16, name="qT_sb", tag="qT_sb")
    kT_sb = pers.tile([128, HPC, T], BF16, name="kT_sb", tag="kT_sb")
    v_sb = pers.tile([128, HPC, NTT, DK], BF16, name="v_sb", tag="v_sb")
    ctxT_sb = pers.tile([128, HPC, T], BF16, name="ctxT_sb", tag="ctxT_sb")

    # ================= Phase 1: q/k/v projections + RoPE =================
    with (
        tc.tile_pool(name="xp", bufs=3) as xp,
        tc.tile_pool(name="p1t", bufs=3) as p1t,
        tc.tile_pool(name="pp_proj", bufs=2, space="PSUM") as pp_proj,
        tc.tile_pool(name="pp_rot", bufs=2, space="PSUM") as pp_rot,
        tc.tile_pool(name="pp_v", bufs=2, space="PSUM") as pp_v,
    ):
        xT_r = xT.rearrange("(n p) t -> p n t", p=128)
        for jb in range(NJB):
            x_t = xp.tile([128, 16, 512], BF16, name="x_t", tag="x_t")
            tb = slice(jb * 512, (jb + 1) * 512)
            for c in range(4):
                nc.sync.dma_start(out=x_t[:, 4 * c:4 * c + 4, :],
                                  in_=xT_r[:, 4 * c:4 * c + 4, tb])
            sblk = (jb % NB) * 512  # position within the batch element
            cos_b = cos_sb[:, sblk:sblk + 512]
            sin_b = sin_sb[:, sblk:sblk + 512]
            for h in range(HPC):
                hs = slice(h * 128, (h + 1) * 128)
                for w_sb, dstT in ((wq_sb, qT_sb), (wk_sb, kT_sb)):
                    ps = pp_proj.tile([128, 512], F32, name="proj_ps", tag="proj_ps")
                    for n in range(16):
                        nc.tensor.matmul(ps, w_sb[:, n, hs], x_t[:, n, :],
                                         start=(n == 0), stop=(n == 15))
                    raw = p1t.tile([128, 512], BF16, name="raw", tag="raw")
                    nc.scalar.copy(out=raw, in_=ps)
                    rot = pp_rot.tile([128, 512], F32, name="rot_ps", tag="rot_ps")
                    nc.tensor.matmul(rot, rT_sb, raw, start=True, stop=True)
                    t1 = p1t.tile([128, 512], F32, name="t1", tag="t1")
                    nc.vector.tensor_mul(out=t1, in0=ps, in1=cos_b)
                    t2 = p1t.tile([128, 512], F32, name="t2", tag="t2")
                    nc.vector.tensor_mul(out=t2, in0=rot, in1=sin_b)
                    nc.vector.tensor_add(out=dstT[:, h, tb], in0=t1, in1=t2)
            for tj in range(4):
                vps = pp_v.tile([128, DH], F32, name="v_ps", tag="v_ps")
                for n in range(16):
                    nc.tensor.matmul(vps, x_t[:, n, tj * 128:(tj + 1) * 128],
                                     wv_sb[:, n, :], start=(n == 0), stop=(n == 15))
                nc.scalar.copy(
                    out=v_sb[:, :, jb * 4 + tj, :],
                    in_=vps.rearrange("p (h k) -> p h k", h=HPC))

    # ======== Phase 2: attention + interleaved output projection =========
    with (
        tc.tile_pool(name="expp", bufs=4) as expp,
        tc.tile_pool(name="accp", bufs=2) as accp,
        tc.tile_pool(name="rdp", bufs=2) as rdp,
        tc.tile_pool(name="rbp", bufs=2) as rbp,
        tc.tile_pool(name="outp", bufs=3) as outp,
        tc.tile_pool(name="ps_sc", bufs=2, space="PSUM") as ps_sc,
        tc.tile_pool(name="ps_ctx", bufs=2, space="PSUM") as ps_ctx,
        tc.tile_pool(name="ps_den", bufs=1, space="PSUM") as ps_den,
        tc.tile_pool(name="ps_bc", bufs=1, space="PSUM") as ps_bc,
        tc.tile_pool(name="ps_o", bufs=2, space="PSUM") as ps_o,
    ):
        for b in range(B):
            for qb in range(NB):
                qg = b * S + qb * 512
                for h in range(HPC):
                    ctx_ps = ps_ctx.tile([128, 512], F32, name="ctx_ps", tag="ctx_ps")
                    acc = accp.tile([128, 512], F32, name="acc", tag="acc")
                    nc.gpsimd.memset(acc, 0.0)
                    nts = 4 * qb + 4
                    for ts in range(nts):
                        j = ts - 4 * qb  # >=0 on diagonal tiles
                        qoff = 128 * j if j >= 0 else 0
                        n_q = 512 - qoff
                        sg = b * S + ts * 128
                        sc = ps_sc.tile([128, 512], F32, name="sc_ps", tag="sc_ps")
                        nc.tensor.matmul(
                            sc[:, :n_q],
                            kT_sb[:, h, sg:sg + 128],
                            qT_sb[:, h, qg + qoff:qg + 512],
                            start=True, stop=True)
                        if j >= 0:
                            nc.vector.tensor_add(
                                out=sc[:, :128], in0=sc[:, :128], in1=negm_sb)
                        ex = expp.tile([128, 512], BF16, name="ex", tag="ex")
                        nc.scalar.activation(
                            out=ex[:, :n_q], in_=sc[:, :n_q], func=EXP, scale=SCALE)
                        nc.vector.tensor_add(
                            out=acc[:, qoff:], in0=acc[:, qoff:], in1=ex[:, :n_q])
                        nc.tensor.matmul(
                            ctx_ps[:, qoff:], v_sb[:, h, b * 16 + ts, :], ex[:, :n_q],
                            start=(ts == 0), stop=(ts == nts - 1))
                    acc_r = accp.tile([128, 512], F32R, name="acc_r", tag="acc_r")
                    nc.vector.tensor_copy(out=acc_r, in_=acc)
                    den_ps = ps_den.tile([1, 512], F32, name="den_ps", tag="den_ps")
                    nc.tensor.matmul(den_ps, onescr_sb, acc_r, start=True, stop=True)
                    rden = rdp.tile([1, 512], F32, name="rden", tag="rden")
                    nc.vector.reciprocal_approx_fast(out=rden, in_=den_ps)
                    rden_r = rdp.tile([1, 512], F32R, name="rden_r", tag="rden_r")
                    nc.vector.tensor_copy(out=rden_r, in_=rden)
                    # broadcast 1/denom across partitions: outer product on PE
                    bc_ps = ps_bc.tile([128, 512], F32, name="bc_ps", tag="bc_ps")
                    nc.tensor.matmul(bc_ps, onesr_sb, rden_r, start=True, stop=True)
                    rbc = rbp.tile([128, 512], F32, name="rbc", tag="rbc")
                    nc.vector.tensor_copy(out=rbc, in_=bc_ps)
                    nc.vector.tensor_mul(
                        out=ctxT_sb[:, h, qg:qg + 512], in0=ctx_ps, in1=rbc)
                # output projection for this (b, qb): both heads ready
                for tt in range(4):
                    i = b * 16 + qb * 4 + tt
                    ot = outp.tile([128, D], BF16, name="ot", tag="ot")
                    for jn in range(4):
                        ops_t = ps_o.tile([128, 512], F32, name="ops", tag="ops")
                        for h in range(HPC):
                            nc.tensor.matmul(
                                ops_t,
                                ctxT_sb[:, h, i * 128:(i + 1) * 128],
                                wo_sb[:, h, jn * 512:(jn + 1) * 512],
                                start=(h == 0), stop=(h == HPC - 1))
                        eng = nc.scalar.copy if jn % 2 == 0 else nc.vector.tensor_copy
                        eng(out=ot[:, jn * 512:(jn + 1) * 512], in_=ops_t)
                    nc.sync.dma_start(out=out[i * 128:(i + 1) * 128, :], in_=ot)


def build_bass():
    nc = bacc.Bacc("TRN2", target_bir_lowering=False, debug=False)
    io = {}

    def din(name, shape, dt):
        io[name] = nc.dram_tensor(name, list(shape), dt, kind="ExternalInput").ap()

    din("xT", (D, T), BF16)
    din("wqT", (D, DH), BF16)
    din("wkT", (D, DH), BF16)
    din("wvT", (D, DH), BF16)
    din("woT", (DH, D), BF16)
    din("cosT", (DK, S), F32)
    din("sinT", (DK, S), F32)
    din("rT", (DK, DK), BF16)
    din("negm", (128, 128), F32)
    din("onescr", (128, 1), F32R)
    din("onesr", (1, 128), F32R)
    io["out"] = nc.dram_tensor("out", [T, D], BF16, kind="ExternalOutput").ap()

    with tile.TileContext(nc) as tc:
        with ExitStack() as ctx:
            _emit(ctx, tc, io)
    nc.compile()
    return nc


def host_tables():
    bf16 = ml_dtypes.bfloat16
    pos = np.arange(S, dtype=np.float64)
    inv = THETA ** (-(np.arange(0, DK, 2, dtype=np.float64) / DK))
    ang = pos[:, None] * inv[None, :]          # [S, 64]
    cosT = np.repeat(np.cos(ang).T, 2, axis=0).astype(np.float32)  # [128, S]
    sinT = np.repeat(np.sin(ang).T, 2, axis=0).astype(np.float32)
    # rotate-half operator: rot = R @ q with rot[2j] = -q[2j+1], rot[2j+1] = q[2j]
    R = np.zeros((DK, DK), np.float32)
    jj = np.arange(0, DK, 2)
    R[jj, jj + 1] = -1.0
    R[jj + 1, jj] = 1.0
    rT = np.ascontiguousarray(R.T).astype(bf16)
    sq = np.arange(128)
    negm = np.where(sq[:, None] > sq[None, :], np.float32(NEGMASK),
                    np.float32(0.0)).astype(np.float32)
    onescr = np.ones((128, 1), np.float32)
    onesr = np.ones((1, 128), np.float32)
    return cosT, sinT, rT, negm, onescr, onesr


def make_in_maps(x, wq, wk, wv, wo):
    bf16 = ml_dtypes.bfloat16
    x = np.asarray(x, np.float32)
    wq, wk, wv, wo = (np.asarray(a, np.float32) for a in (wq, wk, wv, wo))
    xT = np.ascontiguousarray(x.reshape(T, D).T).astype(bf16)
    cosT, sinT, rT, negm, onescr, onesr = host_tables()
    in_maps = []
    for c in range(NCORES):
        hs = slice(c * DH, (c + 1) * DH)
        in_maps.append({
            "xT": xT,
            "wqT": np.ascontiguousarray(wq[hs, :].T).astype(bf16),
            "wkT": np.ascontiguousarray(wk[hs, :].T).astype(bf16),
            "wvT": np.ascontiguousarray(wv[hs, :].T).astype(bf16),
            "woT": np.ascontiguousarray(wo[:, hs].T).astype(bf16),
            "cosT": cosT,
            "sinT": sinT,
            "rT": rT,
            "negm": negm,
            "onescr": onescr,
            "onesr": onesr,
        })
    return in_maps


_CACHE = {}


def get_bass():
    if "nc" not in _CACHE:
        _CACHE["nc"] = build_bass()
    return _CACHE["nc"]


def run(inputs: dict, trace: bool = False):
    """Run on 8 NeuronCores; returns (full_output [B,S,D] f32, BassKernelResults)."""
    nc = get_bass()
    in_maps = make_in_maps(**inputs)
    res = run_bass_kernel_spmd(
        nc, in_maps, core_ids=list(range(NCORES)), trace=trace)
    acc = np.zeros((T, D), np.float32)
    for r in res.results:
        acc += np.asarray(r["out"], dtype=np.float32)
    return acc.reshape(B, S, D), res


def kernel(x, wq, wk, wv, wo):
    out, _ = run(dict(x=x, wq=wq, wk=wk, wv=wv, wo=wo))
    return out


# revision 19
# speedup vs baseline: 1.0896x; 1.0304x over previous
"""Trainium2 Bass kernel: multi-head self-attention with RoPE + causal mask.

Problem shapes (hardcoded): x [2, 2048, 2048] f32, wq/wk/wv/wo [2048, 2048] f32.
  D_MODEL=2048, NUM_HEADS=16, D_K=128, SEQ=2048, BATCH=2, THETA=1e4.

Sharding: tensor-parallel over heads. Each of the 8 cores computes 2 heads:
  - q/k/v projections for its head slice (wq/wk/wv rows 256c:256c+256),
  - RoPE, causal attention (scoresT layout: kv position on the partition dim),
  - its partial output projection (wo columns 256c:256c+256).
The 8 partial outputs [4096, 2048] are summed on the host (the "all-reduce").

Per-core kernel layout choices:
  - qT/kT per head: [dk=128 partitions, 4096 tokens] in SBUF.
  - scoresT tile [128 s, 512 q] so the attention context accumulates with
    N=512 matmuls (lhsT = v tile [128 s, 128 dv], rhs = exp tile).
  - softmax denominator: DVE-accumulated exp sums, reduced across partitions
    with a single f32r ones-column matmul per q-block; reciprocal on DVE;
    broadcast back across partitions via an f32r outer-product matmul.
  - causal mask: additive -1e5 on diagonal 128x128 blocks before exp.
  - output projection interleaved with attention per (batch, q-block) so PE
    has independent work while ScalarE computes exps.
All matmul inputs bf16 (fp32 matmul is 4x slower on TRN2 PE); accumulation f32.
"""

import math
from contextlib import ExitStack

import numpy as np
import ml_dtypes

import concourse.bass as bass
import concourse.tile as tile
from concourse import bacc, mybir
from concourse.bass_utils import run_bass_kernel_spmd

B, S, D, H, DK = 2, 2048, 2048, 16, 128
T = B * S              # 4096 flattened tokens
NCORES = 8
HPC = H // NCORES      # 2 heads per core
DH = HPC * DK          # 256 = per-core head-slice width
THETA = 10000.0
SCALE = 1.0 / math.sqrt(DK)
NEGMASK = -1.0e5

BF16 = mybir.dt.bfloat16
F32 = mybir.dt.float32
F32R = mybir.dt.float32r
EXP = mybir.ActivationFunctionType.Exp

NB = S // 512          # q-blocks per batch element (4)
NJB = T // 512         # token blocks overall (8)
NTT = T // 128         # token tiles overall (32)


def _emit(ctx: ExitStack, tc: tile.TileContext, io: dict):
    nc = tc.nc
    xT, wqT, wkT, wvT, woT = io["xT"], io["wqT"], io["wkT"], io["wvT"], io["woT"]
    out = io["out"]

    const = ctx.enter_context(tc.tile_pool(name="const", bufs=1))
    pers = ctx.enter_context(tc.tile_pool(name="pers", bufs=1))
    xp = ctx.enter_context(tc.tile_pool(name="xp", bufs=3))
    xT_r = xT.rearrange("(n p) t -> p n t", p=128)

    def load_x(jb):
        x_t = xp.tile([128, 16, 512], BF16, name="x_t", tag="x_t")
        for c in range(4):
            nc.sync.dma_start(
                out=x_t[:, 4 * c:4 * c + 4, :],
                in_=xT_r[:, 4 * c:4 * c + 4, jb * 512:(jb + 1) * 512])
        return x_t

    # ---- constants into SBUF, ordered by first use so PE starts early ----
    def load_w(name, src):
        t = const.tile([128, 16, DH], BF16, name=name, tag=name)
        r = src.rearrange("(n p) k -> p n k", p=128)
        for c in range(4):
            nc.sync.dma_start(out=t[:, 4 * c:4 * c + 4, :],
                              in_=r[:, 4 * c:4 * c + 4, :])
        return t

    wq_sb = load_w("wq_sb", wqT)
    x_first = load_x(0)
    wk_sb = load_w("wk_sb", wkT)
    wv_sb = load_w("wv_sb", wvT)
    rT_sb = const.tile([128, 128], BF16, name="rT_sb", tag="rT_sb")
    nc.sync.dma_start(out=rT_sb, in_=io["rT"])
    cos_sb = const.tile([128, S], F32, name="cos_sb", tag="cos_sb")
    nc.sync.dma_start(out=cos_sb, in_=io["cosT"])
    sin_sb = const.tile([128, S], F32, name="sin_sb", tag="sin_sb")
    nc.sync.dma_start(out=sin_sb, in_=io["sinT"])
    negm_sb = const.tile([128, 128], F32, name="negm_sb", tag="negm_sb")
    nc.sync.dma_start(out=negm_sb, in_=io["negm"])
    onescr_sb = const.tile([128, 1], F32R, name="onescr_sb", tag="onescr_sb")
    nc.sync.dma_start(out=onescr_sb, in_=io["onescr"])
    onesr_sb = const.tile([1, 128], F32R, name="onesr_sb", tag="onesr_sb")
    nc.sync.dma_start(out=onesr_sb, in_=io["onesr"])
    wo_sb = const.tile([128, HPC, D], BF16, name="wo_sb", tag="wo_sb")
    nc.sync.dma_start(out=wo_sb, in_=woT.rearrange("(h p) d -> p h d", p=128))

    # ---- persistent per-core activation tensors ----
    qT_sb = pers.tile([128, HPC, T], BF16, name="qT_sb", tag="qT_sb")
    kT_sb = pers.tile([128, HPC, T], BF16, name="kT_sb", tag="kT_sb")
    v_sb = pers.tile([128, HPC, NTT, DK], BF16, name="v_sb", tag="v_sb")
    ctxT_sb = pers.tile([128, HPC, T], BF16, name="ctxT_sb", tag="ctxT_sb")

    # ================= Phase 1: q/k/v projections + RoPE =================
    with (
        tc.tile_pool(name="p1t", bufs=3) as p1t,
        tc.tile_pool(name="pp_proj", bufs=2, space="PSUM") as pp_proj,
        tc.tile_pool(name="pp_rot", bufs=2, space="PSUM") as pp_rot,
        tc.tile_pool(name="pp_v", bufs=2, space="PSUM") as pp_v,
    ):
        for jb in range(NJB):
            x_t = x_first if jb == 0 else load_x(jb)
            tb = slice(jb * 512, (jb + 1) * 512)
            sblk = (jb % NB) * 512  # position within the batch element
            cos_b = cos_sb[:, sblk:sblk + 512]
            sin_b = sin_sb[:, sblk:sblk + 512]
            for h in range(HPC):
                hs = slice(h * 128, (h + 1) * 128)
                for w_sb, dstT in ((wq_sb, qT_sb), (wk_sb, kT_sb)):
                    ps = pp_proj.tile([128, 512], F32, name="proj_ps", tag="proj_ps")
                    for n in range(16):
                        nc.tensor.matmul(ps, w_sb[:, n, hs], x_t[:, n, :],
                                         start=(n == 0), stop=(n == 15))
                    raw = p1t.tile([128, 512], BF16, name="raw", tag="raw")
                    nc.scalar.copy(out=raw, in_=ps)
                    rot = pp_rot.tile([128, 512], F32, name="rot_ps", tag="rot_ps")
                    nc.tensor.matmul(rot, rT_sb, raw, start=True, stop=True)
                    t1 = p1t.tile([128, 512], F32, name="t1", tag="t1")
                    nc.vector.tensor_mul(out=t1, in0=ps, in1=cos_b)
                    t2 = p1t.tile([128, 512], F32, name="t2", tag="t2")
                    nc.vector.tensor_mul(out=t2, in0=rot, in1=sin_b)
                    nc.vector.tensor_add(out=dstT[:, h, tb], in0=t1, in1=t2)
            for tj in range(4):
                vps = pp_v.tile([128, DH], F32, name="v_ps", tag="v_ps")
                for n in range(16):
                    nc.tensor.matmul(vps, x_t[:, n, tj * 128:(tj + 1) * 128],
                                     wv_sb[:, n, :], start=(n == 0), stop=(n == 15))
                nc.scalar.copy(
                    out=v_sb[:, :, jb * 4 + tj, :],
                    in_=vps.rearrange("p (h k) -> p h k", h=HPC))

    # ======== Phase 2: attention + interleaved output projection =========
    with (
        tc.tile_pool(name="expp", bufs=6) as expp,
        tc.tile_pool(name="accp", bufs=2) as accp,
        tc.tile_pool(name="rdp", bufs=2) as rdp,
        tc.tile_pool(name="rbp", bufs=2) as rbp,
        tc.tile_pool(name="outp", bufs=4) as outp,
        tc.tile_pool(name="ps_sc", bufs=2, space="PSUM") as ps_sc,
        tc.tile_pool(name="ps_ctx", bufs=2, space="PSUM") as ps_ctx,
        tc.tile_pool(name="ps_den", bufs=1, space="PSUM") as ps_den,
        tc.tile_pool(name="ps_bc", bufs=1, space="PSUM") as ps_bc,
        tc.tile_pool(name="ps_o", bufs=2, space="PSUM") as ps_o,
    ):
        # Out-projection work for a finished (b, qb) is enqueued as small
        # units and drained between attention matmuls of the NEXT q-block:
        # the PE stream is in-order, so filler work must be emitted at the
        # points where PE would otherwise stall on ScalarE's exp.
        pending = []

        def drain(k):
            for _ in range(k):
                if pending:
                    pending.pop(0)()

        def make_outproj_units(b, qb):
            state = {}
            for tt in range(4):
                i = b * 16 + qb * 4 + tt
                for jn in range(4):
                    def unit(i=i, tt=tt, jn=jn):
                        if jn == 0:
                            state[tt] = outp.tile([128, D], BF16, name="ot",
                                                  tag="ot")
                        ot = state[tt]
                        ops_t = ps_o.tile([128, 512], F32, name="ops", tag="ops")
                        for h in range(HPC):
                            nc.tensor.matmul(
                                ops_t,
                                ctxT_sb[:, h, i * 128:(i + 1) * 128],
                                wo_sb[:, h, jn * 512:(jn + 1) * 512],
                                start=(h == 0), stop=(h == HPC - 1))
                        eng = (nc.scalar.copy if jn % 2 == 0
                               else nc.vector.tensor_copy)
                        eng(out=ot[:, jn * 512:(jn + 1) * 512], in_=ops_t)
                        if jn == 3:
                            nc.sync.dma_start(
                                out=out[i * 128:(i + 1) * 128, :], in_=ot)
                    pending.append(unit)

        for b in range(B):
            for qb in range(NB):
                qg = b * S + qb * 512
                for h in range(HPC):
                    ctx_ps = ps_ctx.tile([128, 512], F32, name="ctx_ps", tag="ctx_ps")
                    acc = accp.tile([128, 512], F32, name="acc", tag="acc")
                    accg = None
                    if qb > 0:
                        accg = accp.tile([128, 512], F32, name="accg", tag="accg")
                    nts = 4 * qb + 4
                    first_v = first_g = True
                    for ts in range(nts):
                        j = ts - 4 * qb  # >=0 on diagonal tiles
                        qoff = 128 * j if j >= 0 else 0
                        n_q = 512 - qoff
                        sg = b * S + ts * 128
                        sc = ps_sc.tile([128, 512], F32, name="sc_ps", tag="sc_ps")
                        nc.tensor.matmul(
                            sc[:, :n_q],
                            kT_sb[:, h, sg:sg + 128],
                            qT_sb[:, h, qg + qoff:qg + 512],
                            start=True, stop=True)
                        if j >= 0:
                            nc.vector.tensor_add(
                                out=sc[:, :128], in0=sc[:, :128], in1=negm_sb)
                        ex = expp.tile([128, 512], BF16, name="ex", tag="ex")
                        nc.scalar.activation(
                            out=ex[:, :n_q], in_=sc[:, :n_q], func=EXP, scale=SCALE)
                        # split exp-sum accumulation DVE/GpSimd by ts parity
                        if accg is not None and ts % 2 == 1:
                            if first_g:
                                nc.gpsimd.tensor_copy(out=accg, in_=ex)
                                first_g = False
                            else:
                                nc.gpsimd.tensor_add(
                                    out=accg[:, qoff:], in0=accg[:, qoff:],
                                    in1=ex[:, :n_q])
                        else:
                            if first_v:
                                nc.vector.tensor_copy(out=acc, in_=ex)
                                first_v = False
                            else:
                                nc.vector.tensor_add(
                                    out=acc[:, qoff:], in0=acc[:, qoff:],
                                    in1=ex[:, :n_q])
                        nc.tensor.matmul(
                            ctx_ps[:, qoff:], v_sb[:, h, b * 16 + ts, :], ex[:, :n_q],
                            start=(ts == 0), stop=(ts == nts - 1))
                        drain(1)
                    if accg is not None:
                        nc.vector.tensor_add(out=acc, in0=acc, in1=accg)
                    acc_r = accp.tile([128, 512], F32R, name="acc_r", tag="acc_r")
                    nc.vector.tensor_copy(out=acc_r, in_=acc)
                    den_ps = ps_den.tile([1, 512], F32, name="den_ps", tag="den_ps")
                    nc.tensor.matmul(den_ps, onescr_sb, acc_r, start=True, stop=True)
                    drain(1)
                    rden = rdp.tile([1, 512], F32, name="rden", tag="rden")
                    nc.vector.reciprocal_approx_fast(out=rden, in_=den_ps)
                    rden_r = rdp.tile([1, 512], F32R, name="rden_r", tag="rden_r")
                    nc.vector.tensor_copy(out=rden_r, in_=rden)
                    # broadcast 1/denom across partitions: outer product on PE
                    bc_ps = ps_bc.tile([128, 512], F32, name="bc_ps", tag="bc_ps")
                    nc.tensor.matmul(bc_ps, onesr_sb, rden_r, start=True, stop=True)
                    drain(1)
                    rbc = rbp.tile([128, 512], F32, name="rbc", tag="rbc")
                    nc.vector.tensor_copy(out=rbc, in_=bc_ps)
                    nc.vector.tensor_mul(
                        out=ctxT_sb[:, h, qg:qg + 512], in0=ctx_ps, in1=rbc)
                drain(len(pending))  # flush any leftovers before enqueueing
                make_outproj_units(b, qb)
        drain(len(pending))


def build_bass():
    nc = bacc.Bacc("TRN2", target_bir_lowering=False, debug=False)
    io = {}

    def din(name, shape, dt):
        io[name] = nc.dram_tensor(name, list(shape), dt, kind="ExternalInput").ap()

    din("xT", (D, T), BF16)
    din("wqT", (D, DH), BF16)
    din("wkT", (D, DH), BF16)
    din("wvT", (D, DH), BF16)
    din("woT", (DH, D), BF16)
    din("cosT", (DK, S), F32)
    din("sinT", (DK, S), F32)
    din("rT", (DK, DK), BF16)
    din("negm", (128, 128), F32)
    din("onescr", (128, 1), F32R)
    din("onesr", (1, 128), F32R)
    io["out"] = nc.dram_tensor("out", [T, D], BF16, kind="ExternalOutput").ap()

    with tile.TileContext(nc) as tc:
        with ExitStack() as ctx:
            _emit(ctx, tc, io)
    nc.compile()
    return nc


def host_tables():
    bf16 = ml_dtypes.bfloat16
    pos = np.arange(S, dtype=np.float64)
    inv = THETA ** (-(np.arange(0, DK, 2, dtype=np.float64) / DK))
    ang = pos[:, None] * inv[None, :]          # [S, 64]
    cosT = np.repeat(np.cos(ang).T, 2, axis=0).astype(np.float32)  # [128, S]
    sinT = np.repeat(np.sin(ang).T, 2, axis=0).astype(np.float32)
    # rotate-half operator: rot = R @ q with rot[2j] = -q[2j+1], rot[2j+1] = q[2j]
    R = np.zeros((DK, DK), np.float32)
    jj = np.arange(0, DK, 2)
    R[jj, jj + 1] = -1.0
    R[jj + 1, jj] = 1.0
    rT = np.ascontiguousarray(R.T).astype(bf16)
    sq = np.arange(128)
    negm = np.where(sq[:, None] > sq[None, :], np.float32(NEGMASK),
                    np.float32(0.0)).astype(np.float32)
    onescr = np.ones((128, 1), np.float32)
    onesr = np.ones((1, 128), np.float32)
    return cosT, sinT, rT, negm, onescr, onesr


def make_in_maps(x, wq, wk, wv, wo):
    bf16 = ml_dtypes.bfloat16
    x = np.asarray(x, np.float32)
    wq, wk, wv, wo = (np.asarray(a, np.float32) for a in (wq, wk, wv, wo))
    xT = np.ascontiguousarray(x.reshape(T, D).T).astype(bf16)
    cosT, sinT, rT, negm, onescr, onesr = host_tables()
    in_maps = []
    for c in range(NCORES):
        hs = slice(c * DH, (c + 1) * DH)
        in_maps.append({
            "xT": xT,
            "wqT": np.ascontiguousarray(wq[hs, :].T).astype(bf16),
            "wkT": np.ascontiguousarray(wk[hs, :].T).astype(bf16),
            "wvT": np.ascontiguousarray(wv[hs, :].T).astype(bf16),
            "woT": np.ascontiguousarray(wo[:, hs].T).astype(bf16),
            "cosT": cosT,
            "sinT": sinT,
            "rT": rT,
            "negm": negm,
            "onescr": onescr,
            "onesr": onesr,
        })
    return in_maps


_CACHE = {}


def get_bass():
    if "nc" not in _CACHE:
        _CACHE["nc"] = build_bass()
    return _CACHE["nc"]


def run(inputs: dict, trace: bool = False):
    """Run on 8 NeuronCores; returns (full_output [B,S,D] f32, BassKernelResults)."""
    nc = get_bass()
    in_maps = make_in_maps(**inputs)
    res = run_bass_kernel_spmd(
        nc, in_maps, core_ids=list(range(NCORES)), trace=trace)
    acc = np.zeros((T, D), np.float32)
    for r in res.results:
        acc += np.asarray(r["out"], dtype=np.float32)
    return acc.reshape(B, S, D), res


def kernel(x, wq, wk, wv, wo):
    out, _ = run(dict(x=x, wq=wq, wk=wk, wv=wv, wo=wo))
    return out


# revision 24
# speedup vs baseline: 1.0924x; 1.0025x over previous
"""Trainium2 Bass kernel: multi-head self-attention with RoPE + causal mask.

Problem shapes (hardcoded): x [2, 2048, 2048] f32, wq/wk/wv/wo [2048, 2048] f32.
  D_MODEL=2048, NUM_HEADS=16, D_K=128, SEQ=2048, BATCH=2, THETA=1e4.

Sharding: tensor-parallel over heads. Each of the 8 cores computes 2 heads:
  - q/k/v projections for its head slice (wq/wk/wv rows 256c:256c+256),
  - RoPE, causal attention (scoresT layout: kv position on the partition dim),
  - its partial output projection (wo columns 256c:256c+256).
The 8 partial outputs [4096, 2048] are summed on the host (the "all-reduce").

Per-core kernel layout choices:
  - qT/kT per head: [dk=128 partitions, 4096 tokens] in SBUF.
  - scoresT tile [128 s, 512 q] so the attention context accumulates with
    N=512 matmuls (lhsT = v tile [128 s, 128 dv], rhs = exp tile).
  - softmax denominator: DVE-accumulated exp sums, reduced across partitions
    with a single f32r ones-column matmul per q-block; reciprocal on DVE;
    broadcast back across partitions via an f32r outer-product matmul.
  - causal mask: additive -1e5 on diagonal 128x128 blocks before exp.
  - output projection interleaved with attention per (batch, q-block) so PE
    has independent work while ScalarE computes exps.
All matmul inputs bf16 (fp32 matmul is 4x slower on TRN2 PE); accumulation f32.
"""

import math
from contextlib import ExitStack

import numpy as np
import ml_dtypes

import concourse.bass as bass
import concourse.tile as tile
from concourse import bacc, mybir
from concourse.bass_utils import run_bass_kernel_spmd

B, S, D, H, DK = 2, 2048, 2048, 16, 128
T = B * S              # 4096 flattened tokens
NCORES = 8
HPC = H // NCORES      # 2 heads per core
DH = HPC * DK          # 256 = per-core head-slice width
THETA = 10000.0
SCALE = 1.0 / math.sqrt(DK)
NEGMASK = -1.0e5

BF16 = mybir.dt.bfloat16
F32 = mybir.dt.float32
F32R = mybir.dt.float32r
EXP = mybir.ActivationFunctionType.Exp

NB = S // 512          # q-blocks per batch element (4)
NJB = T // 512         # token blocks overall (8)
NTT = T // 128         # token tiles overall (32)


def _emit(ctx: ExitStack, tc: tile.TileContext, io: dict):
    nc = tc.nc
    xT, wqT, wkT, wvT, woT = io["xT"], io["wqT"], io["wkT"], io["wvT"], io["woT"]
    out = io["out"]

    const = ctx.enter_context(tc.tile_pool(name="const", bufs=1))
    pers = ctx.enter_context(tc.tile_pool(name="pers", bufs=1))
    xp = tc.alloc_tile_pool(name="xp", bufs=3)  # released after phase 1
    xT_r = xT.rearrange("(n p) t -> p n t", p=128)

    def load_x(jb, chunks=(4, 4, 4, 4)):
        x_t = xp.tile([128, 16, 512], BF16, name="x_t", tag="x_t")
        n0 = 0
        for c in chunks:
            nc.sync.dma_start(
                out=x_t[:, n0:n0 + c, :],
                in_=xT_r[:, n0:n0 + c, jb * 512:(jb + 1) * 512])
            n0 += c
        return x_t

    # ---- constants into SBUF, ordered by first use so PE starts early ----
    def load_w(name, src, chunks=(4, 4, 4, 4)):
        t = const.tile([128, 16, DH], BF16, name=name, tag=name)
        r = src.rearrange("(n p) k -> p n k", p=128)
        n0 = 0
        for c in chunks:
            nc.sync.dma_start(out=t[:, n0:n0 + c, :], in_=r[:, n0:n0 + c, :])
            n0 += c
        return t

    wq_sb = load_w("wq_sb", wqT, chunks=(1, 1, 2, 4, 4, 4))
    x_first = load_x(0, chunks=(1, 1, 2, 4, 4, 4))
    wk_sb = load_w("wk_sb", wkT)
    wv_sb = load_w("wv_sb", wvT)
    rT_sb = const.tile([128, 128], BF16, name="rT_sb", tag="rT_sb")
    nc.sync.dma_start(out=rT_sb, in_=io["rT"])
    cos_sb = const.tile([128, S], F32, name="cos_sb", tag="cos_sb")
    nc.sync.dma_start(out=cos_sb, in_=io["cosT"])
    sin_sb = const.tile([128, S], F32, name="sin_sb", tag="sin_sb")
    nc.sync.dma_start(out=sin_sb, in_=io["sinT"])
    negm_sb = const.tile([128, 128], F32, name="negm_sb", tag="negm_sb")
    nc.sync.dma_start(out=negm_sb, in_=io["negm"])
    onescr_sb = const.tile([128, 1], F32R, name="onescr_sb", tag="onescr_sb")
    nc.sync.dma_start(out=onescr_sb, in_=io["onescr"])
    onesr_sb = const.tile([1, 128], F32R, name="onesr_sb", tag="onesr_sb")
    nc.sync.dma_start(out=onesr_sb, in_=io["onesr"])
    wo_sb = const.tile([128, HPC, D], BF16, name="wo_sb", tag="wo_sb")
    nc.sync.dma_start(out=wo_sb, in_=woT.rearrange("(h p) d -> p h d", p=128))

    # ---- persistent per-core activation tensors ----
    qT_sb = pers.tile([128, HPC, T], BF16, name="qT_sb", tag="qT_sb")
    kT_sb = pers.tile([128, HPC, T], BF16, name="kT_sb", tag="kT_sb")
    v_sb = pers.tile([128, HPC, NTT, DK], BF16, name="v_sb", tag="v_sb")
    ctxT_sb = pers.tile([128, HPC, T], BF16, name="ctxT_sb", tag="ctxT_sb")

    # ================= Phase 1: q/k/v projections + RoPE =================
    with (
        tc.tile_pool(name="p1t", bufs=3) as p1t,
        tc.tile_pool(name="pp_proj", bufs=2, space="PSUM") as pp_proj,
        tc.tile_pool(name="pp_rot", bufs=2, space="PSUM") as pp_rot,
        tc.tile_pool(name="pp_v", bufs=2, space="PSUM") as pp_v,
    ):
        for jb in range(NJB):
            x_t = x_first if jb == 0 else load_x(jb)
            tb = slice(jb * 512, (jb + 1) * 512)
            sblk = (jb % NB) * 512  # position within the batch element
            cos_b = cos_sb[:, sblk:sblk + 512]
            sin_b = sin_sb[:, sblk:sblk + 512]
            for h in range(HPC):
                hs = slice(h * 128, (h + 1) * 128)
                for w_sb, dstT in ((wq_sb, qT_sb), (wk_sb, kT_sb)):
                    ps = pp_proj.tile([128, 512], F32, name="proj_ps", tag="proj_ps")
                    for n in range(16):
                        nc.tensor.matmul(ps, w_sb[:, n, hs], x_t[:, n, :],
                                         start=(n == 0), stop=(n == 15))
                    raw = p1t.tile([128, 512], BF16, name="raw", tag="raw")
                    nc.scalar.copy(out=raw, in_=ps)
                    rot = pp_rot.tile([128, 512], F32, name="rot_ps", tag="rot_ps")
                    nc.tensor.matmul(rot, rT_sb, raw, start=True, stop=True)
                    t1 = p1t.tile([128, 512], F32, name="t1", tag="t1")
                    nc.vector.tensor_mul(out=t1, in0=ps, in1=cos_b)
                    t2 = p1t.tile([128, 512], F32, name="t2", tag="t2")
                    nc.vector.tensor_mul(out=t2, in0=rot, in1=sin_b)
                    nc.vector.tensor_add(out=dstT[:, h, tb], in0=t1, in1=t2)
            for tj in range(4):
                vps = pp_v.tile([128, DH], F32, name="v_ps", tag="v_ps")
                for n in range(16):
                    nc.tensor.matmul(vps, x_t[:, n, tj * 128:(tj + 1) * 128],
                                     wv_sb[:, n, :], start=(n == 0), stop=(n == 15))
                nc.scalar.copy(
                    out=v_sb[:, :, jb * 4 + tj, :],
                    in_=vps.rearrange("p (h k) -> p h k", h=HPC))
    xp.release()

    # ======== Phase 2: attention + interleaved output projection =========
    with (
        tc.tile_pool(name="expp", bufs=6) as expp,
        tc.tile_pool(name="accp", bufs=3) as accp,
        tc.tile_pool(name="rdp", bufs=2) as rdp,
        tc.tile_pool(name="rbp", bufs=2) as rbp,
        tc.tile_pool(name="outp", bufs=4) as outp,
        tc.tile_pool(name="ps_sc", bufs=3, space="PSUM") as ps_sc,
        tc.tile_pool(name="ps_ctx", bufs=2, space="PSUM") as ps_ctx,
        tc.tile_pool(name="ps_den", bufs=1, space="PSUM") as ps_den,
        tc.tile_pool(name="ps_o", bufs=2, space="PSUM") as ps_o,
    ):
        # Out-projection work for a finished (b, qb) is enqueued as small
        # units and drained between attention matmuls of the NEXT q-block:
        # the PE stream is in-order, so filler work must be emitted at the
        # points where PE would otherwise stall on ScalarE's exp.
        pending = []

        def drain(k):
            for _ in range(k):
                if pending:
                    pending.pop(0)()

        def make_outproj_units(b, qb):
            state = {}
            for tt in range(4):
                i = b * 16 + qb * 4 + tt
                for jn in range(4):
                    def unit(i=i, tt=tt, jn=jn):
                        if jn == 0:
                            state[tt] = outp.tile([128, D], BF16, name="ot",
                                                  tag="ot")
                        ot = state[tt]
                        ops_t = ps_o.tile([128, 512], F32, name="ops", tag="ops")
                        for h in range(HPC):
                            nc.tensor.matmul(
                                ops_t,
                                ctxT_sb[:, h, i * 128:(i + 1) * 128],
                                wo_sb[:, h, jn * 512:(jn + 1) * 512],
                                start=(h == 0), stop=(h == HPC - 1))
                        eng = (nc.scalar.copy if jn % 2 == 0
                               else nc.vector.tensor_copy)
                        eng(out=ot[:, jn * 512:(jn + 1) * 512], in_=ops_t)
                        if jn == 3:
                            nc.sync.dma_start(
                                out=out[i * 128:(i + 1) * 128, :], in_=ot)
                    pending.append(unit)

        for b in range(B):
            for qb in range(NB):
                qg = b * S + qb * 512
                nts = 4 * qb + 4
                # both heads advance in lockstep: while ScalarE computes one
                # head's exp, PE runs the other head's matmuls
                ctx_ps, acc, accg, first_v, first_g = {}, {}, {}, {}, {}
                for h in range(HPC):
                    ctx_ps[h] = ps_ctx.tile([128, 512], F32, name="ctx_ps",
                                            tag="ctx_ps")
                    acc[h] = accp.tile([128, 512], F32, name="acc", tag="acc")
                    accg[h] = (accp.tile([128, 512], F32, name="accg", tag="accg")
                               if qb > 0 else None)
                    first_v[h] = first_g[h] = True
                for ts in range(nts):
                    j = ts - 4 * qb  # >=0 on diagonal tiles
                    qoff = 128 * j if j >= 0 else 0
                    n_q = 512 - qoff
                    sg = b * S + ts * 128
                    for h in range(HPC):
                        sc = ps_sc.tile([128, 512], F32, name="sc_ps", tag="sc_ps")
                        nc.tensor.matmul(
                            sc[:, :n_q],
                            kT_sb[:, h, sg:sg + 128],
                            qT_sb[:, h, qg + qoff:qg + 512],
                            start=True, stop=True)
                        if j >= 0:
                            nc.vector.tensor_add(
                                out=sc[:, :128], in0=sc[:, :128], in1=negm_sb)
                        ex = expp.tile([128, 512], BF16, name="ex", tag="ex")
                        nc.scalar.activation(
                            out=ex[:, :n_q], in_=sc[:, :n_q], func=EXP, scale=SCALE)
                        # split exp-sum accumulation DVE/GpSimd by ts parity
                        if accg[h] is not None and ts % 2 == 1:
                            if first_g[h]:
                                nc.gpsimd.tensor_copy(out=accg[h], in_=ex)
                                first_g[h] = False
                            else:
                                nc.gpsimd.tensor_add(
                                    out=accg[h][:, qoff:], in0=accg[h][:, qoff:],
                                    in1=ex[:, :n_q])
                        else:
                            if first_v[h]:
                                nc.vector.tensor_copy(out=acc[h], in_=ex)
                                first_v[h] = False
                            else:
                                nc.vector.tensor_add(
                                    out=acc[h][:, qoff:], in0=acc[h][:, qoff:],
                                    in1=ex[:, :n_q])
                        nc.tensor.matmul(
                            ctx_ps[h][:, qoff:], v_sb[:, h, b * 16 + ts, :],
                            ex[:, :n_q],
                            start=(ts == 0), stop=(ts == nts - 1))
                        drain(1)
                for h in range(HPC):
                    if accg[h] is not None:
                        nc.vector.tensor_add(out=acc[h], in0=acc[h], in1=accg[h])
                    acc_r = accp.tile([128, 512], F32R, name="acc_r", tag="acc_r")
                    nc.vector.tensor_copy(out=acc_r, in_=acc[h])
                    den_ps = ps_den.tile([1, 512], F32, name="den_ps", tag="den_ps")
                    nc.tensor.matmul(den_ps, onescr_sb, acc_r, start=True, stop=True)
                    drain(1)
                    rden = rdp.tile([1, 512], F32, name="rden", tag="rden")
                    nc.vector.reciprocal_approx_fast(out=rden, in_=den_ps)
                    rden_r = rdp.tile([1, 512], F32R, name="rden_r", tag="rden_r")
                    nc.vector.tensor_copy(out=rden_r, in_=rden)
                    # broadcast 1/denom across partitions: outer product on PE,
                    # into a scores-pool slot (saves a PSUM bank)
                    bc_ps = ps_sc.tile([128, 512], F32, name="bc_ps", tag="sc_ps")
                    nc.tensor.matmul(bc_ps, onesr_sb, rden_r, start=True, stop=True)
                    drain(1)
                    rbc = rbp.tile([128, 512], F32, name="rbc", tag="rbc")
                    nc.vector.tensor_copy(out=rbc, in_=bc_ps)
                    nc.vector.tensor_mul(
                        out=ctxT_sb[:, h, qg:qg + 512], in0=ctx_ps[h], in1=rbc)
                    drain(1)
                drain(len(pending))  # flush any leftovers before enqueueing
                make_outproj_units(b, qb)
        drain(len(pending))


def build_bass():
    nc = bacc.Bacc("TRN2", target_bir_lowering=False, debug=False)
    io = {}

    def din(name, shape, dt):
        io[name] = nc.dram_tensor(name, list(shape), dt, kind="ExternalInput").ap()

    din("xT", (D, T), BF16)
    din("wqT", (D, DH), BF16)
    din("wkT", (D, DH), BF16)
    din("wvT", (D, DH), BF16)
    din("woT", (DH, D), BF16)
    din("cosT", (DK, S), F32)
    din("sinT", (DK, S), F32)
    din("rT", (DK, DK), BF16)
    din("negm", (128, 128), F32)
    din("onescr", (128, 1), F32R)
    din("onesr", (1, 128), F32R)
    io["out"] = nc.dram_tensor("out", [T, D], BF16, kind="ExternalOutput").ap()

    with tile.TileContext(nc) as tc:
        with ExitStack() as ctx:
            _emit(ctx, tc, io)
    nc.compile()
    return nc


def host_tables():
    bf16 = ml_dtypes.bfloat16
    pos = np.arange(S, dtype=np.float64)
    inv = THETA ** (-(np.arange(0, DK, 2, dtype=np.float64) / DK))
    ang = pos[:, None] * inv[None, :]          # [S, 64]
    cosT = np.repeat(np.cos(ang).T, 2, axis=0).astype(np.float32)  # [128, S]
    sinT = np.repeat(np.sin(ang).T, 2, axis=0).astype(np.float32)
    # rotate-half operator: rot = R @ q with rot[2j] = -q[2j+1], rot[2j+1] = q[2j]
    R = np.zeros((DK, DK), np.float32)
    jj = np.arange(0, DK, 2)
    R[jj, jj + 1] = -1.0
    R[jj + 1, jj] = 1.0
    rT = np.ascontiguousarray(R.T).astype(bf16)
    sq = np.arange(128)
    negm = np.where(sq[:, None] > sq[None, :], np.float32(NEGMASK),
                    np.float32(0.0)).astype(np.float32)
    onescr = np.ones((128, 1), np.float32)
    onesr = np.ones((1, 128), np.float32)
    return cosT, sinT, rT, negm, onescr, onesr


def make_in_maps(x, wq, wk, wv, wo):
    bf16 = ml_dtypes.bfloat16
    x = np.asarray(x, np.float32)
    wq, wk, wv, wo = (np.asarray(a, np.float32) for a in (wq, wk, wv, wo))
    xT = np.ascontiguousarray(x.reshape(T, D).T).astype(bf16)
    cosT, sinT, rT, negm, onescr, onesr = host_tables()
    in_maps = []
    for c in range(NCORES):
        hs = slice(c * DH, (c + 1) * DH)
        in_maps.append({
            "xT": xT,
            "wqT": np.ascontiguousarray(wq[hs, :].T).astype(bf16),
            "wkT": np.ascontiguousarray(wk[hs, :].T).astype(bf16),
            "wvT": np.ascontiguousarray(wv[hs, :].T).astype(bf16),
            "woT": np.ascontiguousarray(wo[:, hs].T).astype(bf16),
            "cosT": cosT,
            "sinT": sinT,
            "rT": rT,
            "negm": negm,
            "onescr": onescr,
            "onesr": onesr,
        })
    return in_maps


_CACHE = {}


def get_bass():
    if "nc" not in _CACHE:
        _CACHE["nc"] = build_bass()
    return _CACHE["nc"]


def run(inputs: dict, trace: bool = False):
    """Run on 8 NeuronCores; returns (full_output [B,S,D] f32, BassKernelResults)."""
    nc = get_bass()
    in_maps = make_in_maps(**inputs)
    res = run_bass_kernel_spmd(
        nc, in_maps, core_ids=list(range(NCORES)), trace=trace)
    acc = np.zeros((T, D), np.float32)
    for r in res.results:
        acc += np.asarray(r["out"], dtype=np.float32)
    return acc.reshape(B, S, D), res


def kernel(x, wq, wk, wv, wo):
    out, _ = run(dict(x=x, wq=wq, wk=wk, wv=wv, wo=wo))
    return out


# revision 31
# speedup vs baseline: 1.1521x; 1.0547x over previous
"""Trainium2 Bass kernel: multi-head self-attention with RoPE + causal mask.

Problem shapes (hardcoded): x [2, 2048, 2048] f32, wq/wk/wv/wo [2048, 2048] f32.
  D_MODEL=2048, NUM_HEADS=16, D_K=128, SEQ=2048, BATCH=2, THETA=1e4.

Sharding: tensor-parallel over heads. Each of the 8 cores computes 2 heads:
  - q/k/v projections for its head slice (wq/wk/wv rows 256c:256c+256),
  - RoPE, causal attention (scoresT layout: kv position on the partition dim),
  - its partial output projection (wo columns 256c:256c+256).
The 8 partial outputs [4096, 2048] are summed on the host (the "all-reduce").

Per-core kernel layout choices:
  - qT/kT per head: [dk=128 partitions, 4096 tokens] in SBUF.
  - scoresT tile [128 s, 512 q] so the attention context accumulates with
    N=512 matmuls (lhsT = v tile [128 s, 128 dv], rhs = exp tile).
  - softmax denominator: DVE-accumulated exp sums, reduced across partitions
    with a single f32r ones-column matmul per q-block; reciprocal on DVE;
    broadcast back across partitions via an f32r outer-product matmul.
  - causal mask: additive -1e5 on diagonal 128x128 blocks before exp.
  - output projection interleaved with attention per (batch, q-block) so PE
    has independent work while ScalarE computes exps.
All matmul inputs bf16 (fp32 matmul is 4x slower on TRN2 PE); accumulation f32.
"""

import math
from contextlib import ExitStack

import numpy as np
import ml_dtypes

import concourse.bass as bass
import concourse.tile as tile
from concourse import bacc, mybir
from concourse.bass_utils import run_bass_kernel_spmd

B, S, D, H, DK = 2, 2048, 2048, 16, 128
T = B * S              # 4096 flattened tokens
NCORES = 8
HPC = H // NCORES      # 2 heads per core
DH = HPC * DK          # 256 = per-core head-slice width
THETA = 10000.0
SCALE = 1.0 / math.sqrt(DK)
NEGMASK = -1.0e5

BF16 = mybir.dt.bfloat16
F32 = mybir.dt.float32
F32R = mybir.dt.float32r
EXP = mybir.ActivationFunctionType.Exp

NB = S // 512          # q-blocks per batch element (4)
NJB = T // 512         # token blocks overall (8)
NTT = T // 128         # token tiles overall (32)


def _emit(ctx: ExitStack, tc: tile.TileContext, io: dict):
    nc = tc.nc
    xT, wqT, wkT, wvT, woT = io["xT"], io["wqT"], io["wkT"], io["wvT"], io["woT"]
    out = io["out"]

    const = ctx.enter_context(tc.tile_pool(name="const", bufs=1))
    pers = ctx.enter_context(tc.tile_pool(name="pers", bufs=1))
    xp = tc.alloc_tile_pool(name="xp", bufs=3)  # released after phase 1
    xT_r = xT.rearrange("(n p) t -> p n t", p=128)

    def load_x(jb, chunks=(4, 4, 4, 4)):
        x_t = xp.tile([128, 16, 512], BF16, name="x_t", tag="x_t")
        n0 = 0
        for c in chunks:
            nc.sync.dma_start(
                out=x_t[:, n0:n0 + c, :],
                in_=xT_r[:, n0:n0 + c, jb * 512:(jb + 1) * 512])
            n0 += c
        return x_t

    # ---- constants into SBUF, ordered by first use so PE starts early ----
    def load_w(name, src, chunks=(4, 4, 4, 4)):
        t = const.tile([128, 16, DH], BF16, name=name, tag=name)
        r = src.rearrange("(n p) k -> p n k", p=128)
        n0 = 0
        for c in chunks:
            nc.sync.dma_start(out=t[:, n0:n0 + c, :], in_=r[:, n0:n0 + c, :])
            n0 += c
        return t

    wq_sb = load_w("wq_sb", wqT, chunks=(1, 1, 2, 4, 4, 4))
    x_first = load_x(0, chunks=(1, 1, 2, 4, 4, 4))
    wk_sb = load_w("wk_sb", wkT)
    wv_sb = load_w("wv_sb", wvT)
    rT_sb = const.tile([128, 128], BF16, name="rT_sb", tag="rT_sb")
    nc.sync.dma_start(out=rT_sb, in_=io["rT"])
    cos_sb = const.tile([128, S], F32, name="cos_sb", tag="cos_sb")
    nc.sync.dma_start(out=cos_sb, in_=io["cosT"])
    sin_sb = const.tile([128, S], F32, name="sin_sb", tag="sin_sb")
    nc.sync.dma_start(out=sin_sb, in_=io["sinT"])
    negm_sb = const.tile([128, 128], F32, name="negm_sb", tag="negm_sb")
    nc.sync.dma_start(out=negm_sb, in_=io["negm"])
    onesc_sb = const.tile([128, 1], BF16, name="onesc_sb", tag="onesc_sb")
    nc.sync.dma_start(out=onesc_sb, in_=io["onesc"])
    wo_sb = const.tile([128, HPC, D], BF16, name="wo_sb", tag="wo_sb")
    nc.sync.dma_start(out=wo_sb, in_=woT.rearrange("(h p) d -> p h d", p=128))

    # ---- persistent per-core activation tensors ----
    qT_sb = pers.tile([128, HPC, T], BF16, name="qT_sb", tag="qT_sb")
    kT_sb = pers.tile([128, HPC, T], BF16, name="kT_sb", tag="kT_sb")
    v_sb = pers.tile([128, HPC, NTT, DK], BF16, name="v_sb", tag="v_sb")
    ctxT_sb = pers.tile([128, HPC, T], BF16, name="ctxT_sb", tag="ctxT_sb")

    # ================= Phase 1: q/k/v projections + RoPE =================
    with (
        tc.tile_pool(name="p1t", bufs=3) as p1t,
        tc.tile_pool(name="pp_proj", bufs=2, space="PSUM") as pp_proj,
        tc.tile_pool(name="pp_rot", bufs=2, space="PSUM") as pp_rot,
        tc.tile_pool(name="pp_v", bufs=2, space="PSUM") as pp_v,
    ):
        for jb in range(NJB):
            x_t = x_first if jb == 0 else load_x(jb)
            tb = slice(jb * 512, (jb + 1) * 512)
            sblk = (jb % NB) * 512  # position within the batch element
            cos_b = cos_sb[:, sblk:sblk + 512]
            sin_b = sin_sb[:, sblk:sblk + 512]
            for h in range(HPC):
                hs = slice(h * 128, (h + 1) * 128)
                for w_sb, dstT in ((wq_sb, qT_sb), (wk_sb, kT_sb)):
                    ps = pp_proj.tile([128, 512], F32, name="proj_ps", tag="proj_ps")
                    for n in range(16):
                        nc.tensor.matmul(ps, w_sb[:, n, hs], x_t[:, n, :],
                                         start=(n == 0), stop=(n == 15))
                    raw = p1t.tile([128, 512], BF16, name="raw", tag="raw")
                    nc.scalar.copy(out=raw, in_=ps)
                    rot = pp_rot.tile([128, 512], F32, name="rot_ps", tag="rot_ps")
                    nc.tensor.matmul(rot, rT_sb, raw, start=True, stop=True)
                    t1 = p1t.tile([128, 512], F32, name="t1", tag="t1")
                    nc.vector.tensor_mul(out=t1, in0=ps, in1=cos_b)
                    t2 = p1t.tile([128, 512], F32, name="t2", tag="t2")
                    nc.vector.tensor_mul(out=t2, in0=rot, in1=sin_b)
                    nc.vector.tensor_add(out=dstT[:, h, tb], in0=t1, in1=t2)
            for tj in range(4):
                vps = pp_v.tile([128, DH], F32, name="v_ps", tag="v_ps")
                for n in range(16):
                    nc.tensor.matmul(vps, x_t[:, n, tj * 128:(tj + 1) * 128],
                                     wv_sb[:, n, :], start=(n == 0), stop=(n == 15))
                nc.scalar.copy(
                    out=v_sb[:, :, jb * 4 + tj, :],
                    in_=vps.rearrange("p (h k) -> p h k", h=HPC))
    xp.release()

    # ======== Phase 2: attention + interleaved output projection =========
    with (
        tc.tile_pool(name="expp", bufs=6) as expp,
        tc.tile_pool(name="accp", bufs=3) as accp,
        tc.tile_pool(name="crp", bufs=4) as crp,
        tc.tile_pool(name="rdp", bufs=2) as rdp,
        tc.tile_pool(name="rbp", bufs=2) as rbp,
        tc.tile_pool(name="outp", bufs=4) as outp,
        tc.tile_pool(name="dramp", bufs=2, space="DRAM") as dramp,
        tc.tile_pool(name="ps_sc", bufs=3, space="PSUM") as ps_sc,
        tc.tile_pool(name="ps_ctx", bufs=2, space="PSUM") as ps_ctx,
        tc.tile_pool(name="ps_den", bufs=1, space="PSUM") as ps_den,
        tc.tile_pool(name="ps_o", bufs=2, space="PSUM") as ps_o,
    ):
        # Out-projection work for a finished (b, qb) is enqueued as small
        # units and drained between attention matmuls of the NEXT q-block:
        # the PE stream is in-order, so filler work must be emitted at the
        # points where PE would otherwise stall on ScalarE's exp.
        pending = []

        def drain(k):
            for _ in range(k):
                if pending:
                    pending.pop(0)()

        def make_outproj_units(b, qb):
            state = {}
            for tt in range(4):
                i = b * 16 + qb * 4 + tt
                for jn in range(4):
                    def unit(i=i, tt=tt, jn=jn):
                        if jn == 0:
                            state[tt] = outp.tile([128, D], BF16, name="ot",
                                                  tag="ot")
                        ot = state[tt]
                        ops_t = ps_o.tile([128, 512], F32, name="ops", tag="ops")
                        for h in range(HPC):
                            nc.tensor.matmul(
                                ops_t,
                                ctxT_sb[:, h, i * 128:(i + 1) * 128],
                                wo_sb[:, h, jn * 512:(jn + 1) * 512],
                                start=(h == 0), stop=(h == HPC - 1))
                        eng = (nc.scalar.copy if jn % 2 == 0
                               else nc.vector.tensor_copy)
                        eng(out=ot[:, jn * 512:(jn + 1) * 512], in_=ops_t)
                        if jn == 3:
                            nc.sync.dma_start(
                                out=out[i * 128:(i + 1) * 128, :], in_=ot)
                    pending.append(unit)

        for b in range(B):
            for qb in range(NB):
                qg = b * S + qb * 512
                nts = 4 * qb + 4
                # both heads advance in lockstep: while ScalarE computes one
                # head's exp, PE runs the other head's matmuls
                ctx_ps, acc, accg, first_v, first_g = {}, {}, {}, {}, {}
                for h in range(HPC):
                    ctx_ps[h] = ps_ctx.tile([128, 512], F32, name="ctx_ps",
                                            tag="ctx_ps")
                    acc[h] = accp.tile([128, 512], F32, name="acc", tag="acc")
                    accg[h] = (accp.tile([128, 512], F32, name="accg", tag="accg")
                               if qb > 0 else None)
                    first_v[h] = first_g[h] = True
                for ts in range(nts):
                    j = ts - 4 * qb  # >=0 on diagonal tiles
                    qoff = 128 * j if j >= 0 else 0
                    n_q = 512 - qoff
                    sg = b * S + ts * 128
                    for h in range(HPC):
                        sc = ps_sc.tile([128, 512], F32, name="sc_ps", tag="sc_ps")
                        nc.tensor.matmul(
                            sc[:, :n_q],
                            kT_sb[:, h, sg:sg + 128],
                            qT_sb[:, h, qg + qoff:qg + 512],
                            start=True, stop=True)
                        if j >= 0:
                            nc.vector.tensor_add(
                                out=sc[:, :128], in0=sc[:, :128], in1=negm_sb)
                        ex = expp.tile([128, 512], BF16, name="ex", tag="ex")
                        nc.scalar.activation(
                            out=ex[:, :n_q], in_=sc[:, :n_q], func=EXP, scale=SCALE)
                        # split exp-sum accumulation DVE/GpSimd by ts parity
                        if accg[h] is not None and ts % 2 == 1:
                            if first_g[h]:
                                nc.gpsimd.tensor_copy(out=accg[h], in_=ex)
                                first_g[h] = False
                            else:
                                nc.gpsimd.tensor_add(
                                    out=accg[h][:, qoff:], in0=accg[h][:, qoff:],
                                    in1=ex[:, :n_q])
                        else:
                            if first_v[h]:
                                nc.vector.tensor_copy(out=acc[h], in_=ex)
                                first_v[h] = False
                            else:
                                nc.vector.tensor_add(
                                    out=acc[h][:, qoff:], in0=acc[h][:, qoff:],
                                    in1=ex[:, :n_q])
                        nc.tensor.matmul(
                            ctx_ps[h][:, qoff:], v_sb[:, h, b * 16 + ts, :],
                            ex[:, :n_q],
                            start=(ts == 0), stop=(ts == nts - 1))
                        drain(1)
                for h in range(HPC):
                    # free the ctx psum bank right away; normalize later
                    ctx_raw = crp.tile([128, 512], F32, name="ctx_raw",
                                       tag="ctx_raw")
                    nc.scalar.copy(out=ctx_raw, in_=ctx_ps[h])

                    def tail(h=h, a=acc[h], ag=accg[h], ctx_raw=ctx_raw, qg=qg):
                        if ag is not None:
                            nc.vector.tensor_add(out=a, in0=a, in1=ag)
                        acc_bf = accp.tile([128, 512], BF16, name="acc_bf",
                                           tag="acc_bf")
                        nc.vector.tensor_copy(out=acc_bf, in_=a)
                        den_ps = ps_den.tile([1, 512], F32, name="den_ps",
                                             tag="den_ps")
                        nc.tensor.matmul(den_ps, onesc_sb, acc_bf,
                                         start=True, stop=True)
                        rden = rdp.tile([1, 512], F32, name="rden", tag="rden")
                        nc.vector.reciprocal_approx_fast(out=rden, in_=den_ps)
                        # broadcast 1/denom across partitions via a DRAM
                        # bounce: DMA out, then a stride-0 DMA read back
                        dr = dramp.tile([1, 512], F32, name="dr", tag="dr")
                        nc.sync.dma_start(out=dr, in_=rden)
                        rbc = rbp.tile([128, 512], F32, name="rbc", tag="rbc")
                        nc.sync.dma_start(out=rbc, in_=dr.broadcast_to([128, 512]))
                        nc.vector.tensor_mul(
                            out=ctxT_sb[:, h, qg:qg + 512], in0=ctx_raw, in1=rbc)
                    pending.append(tail)
                make_outproj_units(b, qb)
        drain(len(pending))


def build_bass():
    nc = bacc.Bacc("TRN2", target_bir_lowering=False, debug=False)
    io = {}

    def din(name, shape, dt):
        io[name] = nc.dram_tensor(name, list(shape), dt, kind="ExternalInput").ap()

    din("xT", (D, T), BF16)
    din("wqT", (D, DH), BF16)
    din("wkT", (D, DH), BF16)
    din("wvT", (D, DH), BF16)
    din("woT", (DH, D), BF16)
    din("cosT", (DK, S), F32)
    din("sinT", (DK, S), F32)
    din("rT", (DK, DK), BF16)
    din("negm", (128, 128), F32)
    din("onesc", (128, 1), BF16)
    io["out"] = nc.dram_tensor("out", [T, D], BF16, kind="ExternalOutput").ap()

    with tile.TileContext(nc) as tc:
        with ExitStack() as ctx:
            _emit(ctx, tc, io)
    nc.compile()
    return nc


def host_tables():
    bf16 = ml_dtypes.bfloat16
    pos = np.arange(S, dtype=np.float64)
    inv = THETA ** (-(np.arange(0, DK, 2, dtype=np.float64) / DK))
    ang = pos[:, None] * inv[None, :]          # [S, 64]
    cosT = np.repeat(np.cos(ang).T, 2, axis=0).astype(np.float32)  # [128, S]
    sinT = np.repeat(np.sin(ang).T, 2, axis=0).astype(np.float32)
    # rotate-half operator: rot = R @ q with rot[2j] = -q[2j+1], rot[2j+1] = q[2j]
    R = np.zeros((DK, DK), np.float32)
    jj = np.arange(0, DK, 2)
    R[jj, jj + 1] = -1.0
    R[jj + 1, jj] = 1.0
    rT = np.ascontiguousarray(R.T).astype(bf16)
    sq = np.arange(128)
    negm = np.where(sq[:, None] > sq[None, :], np.float32(NEGMASK),
                    np.float32(0.0)).astype(np.float32)
    onesc = np.ones((128, 1), ml_dtypes.bfloat16)
    return cosT, sinT, rT, negm, onesc


def make_in_maps(x, wq, wk, wv, wo):
    bf16 = ml_dtypes.bfloat16
    x = np.asarray(x, np.float32)
    wq, wk, wv, wo = (np.asarray(a, np.float32) for a in (wq, wk, wv, wo))
    xT = np.ascontiguousarray(x.reshape(T, D).T).astype(bf16)
    cosT, sinT, rT, negm, onesc = host_tables()
    in_maps = []
    for c in range(NCORES):
        hs = slice(c * DH, (c + 1) * DH)
        in_maps.append({
            "xT": xT,
            "wqT": np.ascontiguousarray(wq[hs, :].T).astype(bf16),
            "wkT": np.ascontiguousarray(wk[hs, :].T).astype(bf16),
            "wvT": np.ascontiguousarray(wv[hs, :].T).astype(bf16),
            "woT": np.ascontiguousarray(wo[:, hs].T).astype(bf16),
            "cosT": cosT,
            "sinT": sinT,
            "rT": rT,
            "negm": negm,
            "onesc": onesc,
        })
    return in_maps


_CACHE = {}


def get_bass():
    if "nc" not in _CACHE:
        _CACHE["nc"] = build_bass()
    return _CACHE["nc"]


def run(inputs: dict, trace: bool = False):
    """Run on 8 NeuronCores; returns (full_output [B,S,D] f32, BassKernelResults)."""
    nc = get_bass()
    in_maps = make_in_maps(**inputs)
    res = run_bass_kernel_spmd(
        nc, in_maps, core_ids=list(range(NCORES)), trace=trace)
    acc = np.zeros((T, D), np.float32)
    for r in res.results:
        acc += np.asarray(r["out"], dtype=np.float32)
    return acc.reshape(B, S, D), res


def kernel(x, wq, wk, wv, wo):
    out, _ = run(dict(x=x, wq=wq, wk=wk, wv=wv, wo=wo))
    return out
